# revision 13
# baseline (speedup 1.0000x reference)
"""BertSelfAttention TRN2 kernel.

Full inputs -> head-sharded across 8 NeuronCores (2 heads / core, batch
replicated) -> full [B, S, H] output.

Shapes (hardcoded): B=2, S=2048, H=1024, NH=16, DH=64.

Per-core plan (features = this core's 2 heads x 64 dims = 128):
  phase 0: weight slices loaded block-swizzled from DRAM + DVE 32x32
           stream-transpose -> wT [k, f] fp16 chunks (no PE transposes
           anywhere in this kernel: the 4-byte / transpose matmul path
           only supports one sync-wait on its LDWEIGHTS struct, which
           Tile regularly exceeds).
  phase 1: per 512-token slab, hs loaded block-swizzled + DVE
           stream-transpose -> hsT [k, t] fp16; fp16 matmuls with wT
           accumulate Q^T/K^T/V^T [128 f, t] in fp32 PSUM; bias added
           during the PSUM->SBUF copy (output fp16).
  phase 1.5: V_aug [ktok, 65] per (batch, head) = [V * mask | mask] via
           2-byte xbar DMA transpose of V^T chunks -- folds the
           attention mask and the softmax denominator into the context
           matmul.
  phase 2: per (batch, qblock of 512), per k-chunk of 128 tokens:
           S^T [128 ktok, 2x512 q] = K^T.T @ Q^T, both heads packed in
           one PSUM tile (PE row tiling, K=64 per head);
           P^T = exp(0.125 * S^T): one ACT op covering both heads (fp16
           out); ctx^T [65, 512 q] += V_aug.T @ P^T accumulated over
           k-chunks in fp32 PSUM; ctx^T copied to fp16 (padded to 80
           rows), xbar-DMA-transposed to [q, 80], divided by the
           denominator column, written out in fp32.

Mask handling: probs = exp(s)*m / sum_k exp(s)*m, which equals
softmax(s + (1-m)*-1e4) exactly for binary masks (the reference's -1e4
bias underflows exp to exactly 0 in fp32).
"""

import sys

sys.path.insert(0, "/opt/trn_rl_repo")

import numpy as np

import concourse.bass as bass
import concourse.mybir as mybir
import concourse.tile as tile
from concourse.bass import ds, ts

P = 128
B = 2
S = 2048
HID = 1024
NH = 16
DH = 64
T = B * S  # 4096 tokens
NCORES = 8
FPC = 128  # features per core (2 heads x 64)
NHL = 2  # local heads
KCH = HID // P  # 8 contraction chunks for projections
SLAB = 512
NSLAB = T // SLAB  # 8
SKC = S // P  # 16 k-token chunks per sequence
QB = 512
NQB = S // QB  # 4
CPAD = 80  # ctx rows padded to multiple of 16 for xbar transpose
F32 = mybir.dt.float32
F16 = mybir.dt.float16


def _load_cast_transpose(nc, pool_x, dsts, dram_ap, row0, tag):
    """Load dram_ap[row0:row0+128, :1024] (f32), cast to fp16, and xbar-
    transpose each 128-col chunk j into dsts[j] (a [128, >=128] fp16 AP
    slice to receive chunk j transposed)."""
    x = pool_x.tile([P, HID], F32, tag=tag, name=tag)
    nc.sync.dma_start(x, dram_ap[ds(row0, P), :])
    xh = pool_x.tile([P, HID], F16, tag=tag + "h", name=tag + "h")
    nc.vector.tensor_copy(xh, x)
    for j, d in enumerate(dsts):
        nc.sync.dma_start_transpose(d, xh[:, ts(j, P)])


def _split_waits(nc):
    """This walrus build only supports one sync-wait per instruction;
    move extra waits onto preceding same-engine NoOps (sequencer-level,
    order-preserving)."""
    for fn in nc.m.functions:
        for blk in fn.blocks:
            out = []
            for inst in blk.instructions:
                si = inst.sync_info
                ow = list((si.on_wait if si else None) or [])
                if len(ow) >= 2:
                    for k, w in enumerate(ow[:-1]):
                        out.append(
                            mybir.InstNoOp(
                                name=f"{inst.name}-wait{k}",
                                engine=inst.engine,
                                bass_nofuse=True,
                                sync_info=mybir.SyncInfo(
                                    on_wait=[w], on_update=[]
                                ),
                            )
                        )
                    inst.sync_info = mybir.SyncInfo(
                        on_wait=[ow[-1]], on_update=si.on_update
                    )
                out.append(inst)
            blk.instructions[:] = out
    return nc


def _build_program():
    nc = bass.Bass()

    hs_d = nc.declare_dram_parameter("hs", [T, HID], F32, isOutput=False)
    w_d = {
        p: nc.declare_dram_parameter(f"w{p}", [FPC, HID], F32, isOutput=False)
        for p in "qkv"
    }
    b_d = {
        p: nc.declare_dram_parameter(f"b{p}", [FPC], F32, isOutput=False)
        for p in "qkv"
    }
    # mask_t[p, b*SKC + kc] = attention_mask[b, kc*128 + p]
    mk_d = nc.declare_dram_parameter("mk", [P, B * SKC], F32, isOutput=False)
    out_d = nc.declare_dram_parameter("out", [T, FPC], F32, isOutput=True)

    with tile.TileContext(nc) as tc:
        _emit(nc, tc, hs_d, w_d, b_d, mk_d, out_d)
    return _split_waits(nc)


def _emit(nc, tc, hs_d, w_d, b_d, mk_d, out_d):
    from contextlib import ExitStack

    with ExitStack() as ctx:
        consts = ctx.enter_context(tc.tile_pool(name="consts", bufs=1))
        wpool = ctx.enter_context(tc.tile_pool(name="wpool", bufs=1))
        xpool = ctx.enter_context(tc.tile_pool(name="xpool", bufs=2))
        hstp = ctx.enter_context(tc.tile_pool(name="hstp", bufs=2))
        qkvp = ctx.enter_context(tc.tile_pool(name="qkvp", bufs=1))
        vaugp = ctx.enter_context(tc.tile_pool(name="vaugp", bufs=1))
        ptp = ctx.enter_context(tc.tile_pool(name="ptp", bufs=3))
        ctxsb = ctx.enter_context(tc.tile_pool(name="ctxsb", bufs=2))
        potp = ctx.enter_context(tc.tile_pool(name="potp", bufs=4))
        outp = ctx.enter_context(tc.tile_pool(name="outp", bufs=8))
        smallp = ctx.enter_context(tc.tile_pool(name="smallp", bufs=4))
        # PSUM budget (8 banks): proj 2 + scores 2x2 + ctx 2 = 8
        ps_proj = ctx.enter_context(
            tc.tile_pool(name="ps_proj", bufs=2, space="PSUM")
        )
        ps_sc = ctx.enter_context(
            tc.tile_pool(name="ps_sc", bufs=2, space="PSUM")
        )
        ps_ctx = ctx.enter_context(
            tc.tile_pool(name="ps_ctx", bufs=1, space="PSUM")
        )

        # ---- phase 0: constants, weights (swizzle + DVE transpose) ----
        mask_sb = consts.tile([P, B * SKC], F32)
        nc.sync.dma_start(mask_sb, mk_d[:, :])

        bias_sb = {}
        for p in "qkv":
            bias_sb[p] = consts.tile(
                [P, 1], F32, tag=f"bias_{p}", name=f"bias_{p}"
            )
            nc.sync.dma_start(bias_sb[p], b_d[p].rearrange("(p o) -> p o", o=1))

        wT = {}
        for p in "qkv":
            for j in range(KCH):
                wT[(p, j)] = wpool.tile(
                    [P, P], F16, tag=f"wT_{p}{j}", name=f"wT_{p}{j}"
                )
            _load_cast_transpose(
                nc, xpool, [wT[(p, j)] for j in range(KCH)], w_d[p], 0,
                tag=f"wx_{p}",
            )

        # ---- phase 1: hsT + projections, per slab ----
        qT, kT, vT = {}, {}, {}
        for b in range(B):
            for j in range(NQB):
                qT[(b, j)] = qkvp.tile(
                    [P, SLAB], F16, tag=f"qT{b}_{j}", name=f"qT{b}_{j}"
                )
                kT[(b, j)] = qkvp.tile(
                    [P, SLAB], F16, tag=f"kT{b}_{j}", name=f"kT{b}_{j}"
                )
                vT[(b, j)] = qkvp.tile(
                    [P, SLAB], F16, tag=f"vT{b}_{j}", name=f"vT{b}_{j}"
                )
        sb_of = {"q": qT, "k": kT, "v": vT}

        for sl in range(NSLAB):
            b = (sl * SLAB) // S
            jq = (sl * SLAB) % S // SLAB  # quarter index within batch
            hsT = [
                hstp.tile([P, SLAB], F16, tag=f"hsT{kc}", name=f"hsT{kc}")
                for kc in range(KCH)
            ]
            for tq in range(SLAB // P):
                _load_cast_transpose(
                    nc,
                    xpool,
                    [hsT[kc][:, ts(tq, P)] for kc in range(KCH)],
                    hs_d,
                    sl * SLAB + tq * P,
                    tag=f"x{tq}",
                )

            for p in "qkv":
                acc = ps_proj.tile([P, SLAB], F32, tag="proj")
                for kc in range(KCH):
                    nc.tensor.matmul(
                        acc,
                        wT[(p, kc)],
                        hsT[kc],
                        start=(kc == 0),
                        stop=(kc == KCH - 1),
                    )
                nc.vector.tensor_scalar_add(sb_of[p][(b, jq)], acc, bias_sb[p])

        # ---- phase 1.5: V_aug per (b, h, kc): [128 ktok, DH+1] fp16 ----
        # xbar transpose of V^T chunk, then mask-scale in place
        vaug = {}
        for b in range(B):
            for kc in range(SKC):
                mcol = mask_sb[:, ds(b * SKC + kc, 1)]
                for h in range(NHL):
                    va = vaugp.tile(
                        [P, DH + 1],
                        F16,
                        tag=f"vaug{b}{h}_{kc}",
                        name=f"vaug{b}{h}_{kc}",
                    )
                    vaug[(b, h, kc)] = va
                    nc.sync.dma_start_transpose(
                        va[:, :DH],
                        vT[(b, kc // 4)][ds(h * DH, DH), ts(kc % 4, P)],
                    )
                    nc.vector.tensor_scalar_mul(va[:, :DH], va[:, :DH], mcol)
                    nc.vector.tensor_copy(va[:, DH : DH + 1], mcol)

        # ---- phase 2: attention ----
        for b in range(B):
            for qb in range(NQB):
                cps_full = [
                    ps_ctx.tile([P, QB], F32, tag=f"ctx{h}", name=f"ctx{h}")
                    for h in range(NHL)
                ]
                cps = [c[: DH + 1] for c in cps_full]
                for kc in range(SKC):
                    scp = ps_sc.tile([P, NHL * QB], F32, tag="sc")
                    for h in range(NHL):
                        nc.tensor.matmul(
                            scp[:, ts(h, QB)],
                            kT[(b, kc // 4)][
                                ds(h * DH, DH), ds((kc % 4) * P, P)
                            ],
                            qT[(b, qb)][ds(h * DH, DH), :],
                            start=True,
                            stop=True,
                            tile_position=(h * DH, 0),
                        )
                    pt = ptp.tile([P, NHL * QB], F16, tag="pT")
                    nc.scalar.activation(
                        pt, scp, mybir.ActivationFunctionType.Exp, scale=0.125
                    )
                    for h in range(NHL):
                        nc.tensor.matmul(
                            cps[h],
                            vaug[(b, h, kc)],
                            pt[:, ts(h, QB)],
                            start=(kc == 0),
                            stop=(kc == SKC - 1),
                        )
                # normalize + transpose out (xbar transpose, fp16 staging)
                out_tiles = [
                    outp.tile([P, FPC], F32, tag="out", name=f"out{i}")
                    for i in range(QB // P)
                ]
                for h in range(NHL):
                    csb = ctxsb.tile(
                        [CPAD, QB], F16, tag=f"csb{h}", name=f"csb{h}"
                    )
                    nc.vector.memset(csb, 1.0)
                    nc.vector.tensor_copy(csb[: DH + 1], cps[h])
                    for tb in range(QB // P):
                        pot = potp.tile([P, CPAD], F16, tag="pot", name="pot")
                        nc.sync.dma_start_transpose(pot, csb[:, ts(tb, P)])
                        rec = smallp.tile([P, 1], F32, tag="rec", name="rec")
                        nc.vector.reciprocal(rec, pot[:, DH : DH + 1])
                        nc.vector.tensor_scalar_mul(
                            out_tiles[tb][:, ds(h * DH, DH)], pot[:, :DH], rec
                        )
                for tb in range(QB // P):
                    nc.sync.dma_start(
                        out_d[ds(b * S + qb * QB + tb * P, P), :],
                        out_tiles[tb],
                    )


_nc = None


def _get_program():
    global _nc
    if _nc is None:
        _nc = _build_program()
    return _nc


def kernel(
    hidden_states,
    attention_mask,
    Wq,
    bq,
    Wk,
    bk,
    Wv,
    bv,
):
    from concourse.bass_utils import run_bass_kernel_spmd

    hs = np.ascontiguousarray(np.asarray(hidden_states, np.float32)).reshape(
        T, HID
    )
    mask = np.asarray(attention_mask, np.float32)
    mask_t = np.ascontiguousarray(
        mask.reshape(B, SKC, P).transpose(2, 0, 1).reshape(P, B * SKC)
    )
    Wq, Wk, Wv = (np.asarray(w, np.float32) for w in (Wq, Wk, Wv))
    bq, bk, bv = (np.asarray(x, np.float32) for x in (bq, bk, bv))

    nc = _get_program()
    in_maps = []
    for c in range(NCORES):
        sl = slice(c * FPC, (c + 1) * FPC)
        in_maps.append(
            {
                "hs": hs,
                "wq": np.ascontiguousarray(Wq[sl]),
                "wk": np.ascontiguousarray(Wk[sl]),
                "wv": np.ascontiguousarray(Wv[sl]),
                "bq": np.ascontiguousarray(bq[sl]),
                "bk": np.ascontiguousarray(bk[sl]),
                "bv": np.ascontiguousarray(bv[sl]),
                "mk": mask_t,
            }
        )
    res = run_bass_kernel_spmd(nc, in_maps, list(range(NCORES))).results
    full = np.concatenate(
        [np.asarray(res[c]["out"]) for c in range(NCORES)], axis=1
    )
    return full.reshape(B, S, NH * DH).astype(np.float32)


# revision 14
# speedup vs baseline: 1.2502x; 1.2502x over previous
"""BertSelfAttention TRN2 kernel.

Full inputs -> head-sharded across 8 NeuronCores (2 heads / core, batch
replicated) -> full [B, S, H] output.

Shapes (hardcoded): B=2, S=2048, H=1024, NH=16, DH=64.

Per-core plan (features = this core's 2 heads x 64 dims = 128):
  phase 0: weight slices loaded block-swizzled from DRAM + DVE 32x32
           stream-transpose -> wT [k, f] fp16 chunks (no PE transposes
           anywhere in this kernel: the 4-byte / transpose matmul path
           only supports one sync-wait on its LDWEIGHTS struct, which
           Tile regularly exceeds).
  phase 1: per 512-token slab, hs loaded block-swizzled + DVE
           stream-transpose -> hsT [k, t] fp16; fp16 matmuls with wT
           accumulate Q^T/K^T/V^T [128 f, t] in fp32 PSUM; bias added
           during the PSUM->SBUF copy (output fp16).
  phase 1.5: V_aug [ktok, 65] per (batch, head) = [V * mask | mask] via
           2-byte xbar DMA transpose of V^T chunks -- folds the
           attention mask and the softmax denominator into the context
           matmul.
  phase 2: per (batch, qblock of 512), per k-chunk of 128 tokens:
           S^T [128 ktok, 2x512 q] = K^T.T @ Q^T, both heads packed in
           one PSUM tile (PE row tiling, K=64 per head);
           P^T = exp(0.125 * S^T): one ACT op covering both heads (fp16
           out); ctx^T [65, 512 q] += V_aug.T @ P^T accumulated over
           k-chunks in fp32 PSUM; ctx^T copied to fp16 (padded to 80
           rows), xbar-DMA-transposed to [q, 80], divided by the
           denominator column, written out in fp32.

Mask handling: probs = exp(s)*m / sum_k exp(s)*m, which equals
softmax(s + (1-m)*-1e4) exactly for binary masks (the reference's -1e4
bias underflows exp to exactly 0 in fp32).
"""

import sys

sys.path.insert(0, "/opt/trn_rl_repo")

import os

import numpy as np

import concourse.bass as bass
import concourse.mybir as mybir
import concourse.tile as tile
from concourse.bass import ds, ts

P = 128
B = 2
S = 2048
HID = 1024
NH = 16
DH = 64
T = B * S  # 4096 tokens
NCORES = 8
FPC = 128  # features per core (2 heads x 64)
NHL = 2  # local heads
KCH = HID // P  # 8 contraction chunks for projections
SLAB = 512
NSLAB = T // SLAB  # 8
SKC = S // P  # 16 k-token chunks per sequence
QB = 512
NQB = S // QB  # 4
CPAD = 80  # ctx rows padded to multiple of 16 for xbar transpose
F32 = mybir.dt.float32
F16 = mybir.dt.float16
NO_XBAR = bool(int(os.environ.get("K_NO_XBAR", "0")))


def _xbar_or_fake(nc, dst, src_ap, fake_src):
    if NO_XBAR:
        nc.vector.tensor_copy(dst, fake_src)
    else:
        nc.sync.dma_start_transpose(dst, src_ap)


def _load_cast_transpose(nc, pool_x, dsts, dram_ap, row0, tag):
    """Load dram_ap[row0:row0+128, :1024] (f32), cast to fp16, and xbar-
    transpose each 128-col chunk j into dsts[j] (a [128, >=128] fp16 AP
    slice to receive chunk j transposed)."""
    x = pool_x.tile([P, HID], F32, tag=tag, name=tag)
    nc.sync.dma_start(x, dram_ap[ds(row0, P), :])
    xh = pool_x.tile([P, HID], F16, tag=tag + "h", name=tag + "h")
    nc.vector.tensor_copy(xh, x)
    for j, d in enumerate(dsts):
        _xbar_or_fake(nc, d, xh[:, ts(j, P)], xh[:, ts(j, P)][:, : d.shape[-1]])


def _split_waits(nc):
    """This walrus build only supports one sync-wait per instruction;
    move extra waits onto preceding same-engine NoOps (sequencer-level,
    order-preserving)."""
    for fn in nc.m.functions:
        for blk in fn.blocks:
            out = []
            for inst in blk.instructions:
                si = inst.sync_info
                ow = list((si.on_wait if si else None) or [])
                if len(ow) >= 2:
                    for k, w in enumerate(ow[:-1]):
                        out.append(
                            mybir.InstNoOp(
                                name=f"{inst.name}-wait{k}",
                                engine=inst.engine,
                                bass_nofuse=True,
                                sync_info=mybir.SyncInfo(
                                    on_wait=[w], on_update=[]
                                ),
                            )
                        )
                    inst.sync_info = mybir.SyncInfo(
                        on_wait=[ow[-1]], on_update=si.on_update
                    )
                out.append(inst)
            blk.instructions[:] = out
    return nc


def _build_program():
    nc = bass.Bass()

    hs_d = nc.declare_dram_parameter("hs", [T, HID], F32, isOutput=False)
    w_d = {
        p: nc.declare_dram_parameter(f"w{p}", [FPC, HID], F32, isOutput=False)
        for p in "qkv"
    }
    b_d = {
        p: nc.declare_dram_parameter(f"b{p}", [FPC], F32, isOutput=False)
        for p in "qkv"
    }
    # mask_t[p, b*SKC + kc] = attention_mask[b, kc*128 + p]
    mk_d = nc.declare_dram_parameter("mk", [P, B * SKC], F32, isOutput=False)
    out_d = nc.declare_dram_parameter("out", [T, FPC], F32, isOutput=True)

    with tile.TileContext(nc) as tc:
        _emit(nc, tc, hs_d, w_d, b_d, mk_d, out_d)
    return _split_waits(nc)


def _emit(nc, tc, hs_d, w_d, b_d, mk_d, out_d):
    from contextlib import ExitStack

    with ExitStack() as ctx:
        consts = ctx.enter_context(tc.tile_pool(name="consts", bufs=1))
        wpool = ctx.enter_context(tc.tile_pool(name="wpool", bufs=1))
        xpool = ctx.enter_context(tc.tile_pool(name="xpool", bufs=2))
        hstp = ctx.enter_context(tc.tile_pool(name="hstp", bufs=2))
        qkvp = ctx.enter_context(tc.tile_pool(name="qkvp", bufs=1))
        vaugp = ctx.enter_context(tc.tile_pool(name="vaugp", bufs=1))
        ptp = ctx.enter_context(tc.tile_pool(name="ptp", bufs=3))
        ctxsb = ctx.enter_context(tc.tile_pool(name="ctxsb", bufs=2))
        potp = ctx.enter_context(tc.tile_pool(name="potp", bufs=4))
        outp = ctx.enter_context(tc.tile_pool(name="outp", bufs=8))
        smallp = ctx.enter_context(tc.tile_pool(name="smallp", bufs=4))
        # PSUM budget (8 banks): proj 2 + scores 2x2 + ctx 2 = 8
        ps_proj = ctx.enter_context(
            tc.tile_pool(name="ps_proj", bufs=2, space="PSUM")
        )
        ps_sc = ctx.enter_context(
            tc.tile_pool(name="ps_sc", bufs=2, space="PSUM")
        )
        ps_ctx = ctx.enter_context(
            tc.tile_pool(name="ps_ctx", bufs=1, space="PSUM")
        )

        # ---- phase 0: constants, weights (swizzle + DVE transpose) ----
        mask_sb = consts.tile([P, B * SKC], F32)
        nc.sync.dma_start(mask_sb, mk_d[:, :])

        bias_sb = {}
        for p in "qkv":
            bias_sb[p] = consts.tile(
                [P, 1], F32, tag=f"bias_{p}", name=f"bias_{p}"
            )
            nc.sync.dma_start(bias_sb[p], b_d[p].rearrange("(p o) -> p o", o=1))

        wT = {}
        for p in "qkv":
            for j in range(KCH):
                wT[(p, j)] = wpool.tile(
                    [P, P], F16, tag=f"wT_{p}{j}", name=f"wT_{p}{j}"
                )
            _load_cast_transpose(
                nc, xpool, [wT[(p, j)] for j in range(KCH)], w_d[p], 0,
                tag=f"wx_{p}",
            )

        # ---- phase 1: hsT + projections, per slab ----
        qT, kT, vT = {}, {}, {}
        for b in range(B):
            for j in range(NQB):
                qT[(b, j)] = qkvp.tile(
                    [P, SLAB], F16, tag=f"qT{b}_{j}", name=f"qT{b}_{j}"
                )
                kT[(b, j)] = qkvp.tile(
                    [P, SLAB], F16, tag=f"kT{b}_{j}", name=f"kT{b}_{j}"
                )
                vT[(b, j)] = qkvp.tile(
                    [P, SLAB], F16, tag=f"vT{b}_{j}", name=f"vT{b}_{j}"
                )
        sb_of = {"q": qT, "k": kT, "v": vT}

        for sl in range(NSLAB):
            b = (sl * SLAB) // S
            jq = (sl * SLAB) % S // SLAB  # quarter index within batch
            hsT = [
                hstp.tile([P, SLAB], F16, tag=f"hsT{kc}", name=f"hsT{kc}")
                for kc in range(KCH)
            ]
            for tq in range(SLAB // P):
                _load_cast_transpose(
                    nc,
                    xpool,
                    [hsT[kc][:, ts(tq, P)] for kc in range(KCH)],
                    hs_d,
                    sl * SLAB + tq * P,
                    tag=f"x{tq}",
                )

            for p in "qkv":
                acc = ps_proj.tile([P, SLAB], F32, tag="proj")
                for kc in range(KCH):
                    nc.tensor.matmul(
                        acc,
                        wT[(p, kc)],
                        hsT[kc],
                        start=(kc == 0),
                        stop=(kc == KCH - 1),
                    )
                nc.vector.tensor_scalar_add(sb_of[p][(b, jq)], acc, bias_sb[p])

        # ---- phase 1.5: V_aug per (b, h, kc): [128 ktok, DH+1] fp16 ----
        # xbar transpose of V^T chunk, then mask-scale in place
        vaug = {}
        for b in range(B):
            for kc in range(SKC):
                mcol = mask_sb[:, ds(b * SKC + kc, 1)]
                for h in range(NHL):
                    va = vaugp.tile(
                        [P, DH + 1],
                        F16,
                        tag=f"vaug{b}{h}_{kc}",
                        name=f"vaug{b}{h}_{kc}",
                    )
                    vaug[(b, h, kc)] = va
                    _xbar_or_fake(
                        nc,
                        va[:, :DH],
                        vT[(b, kc // 4)][ds(h * DH, DH), ts(kc % 4, P)],
                        vT[(b, kc // 4)][:, :DH],
                    )
                    nc.vector.tensor_scalar_mul(va[:, :DH], va[:, :DH], mcol)
                    nc.vector.tensor_copy(va[:, DH : DH + 1], mcol)

        # ---- phase 2: attention ----
        for b in range(B):
            for qb in range(NQB):
                cps_full = [
                    ps_ctx.tile([P, QB], F32, tag=f"ctx{h}", name=f"ctx{h}")
                    for h in range(NHL)
                ]
                cps = [c[: DH + 1] for c in cps_full]
                for kc in range(SKC):
                    scp = ps_sc.tile([P, NHL * QB], F32, tag="sc")
                    for h in range(NHL):
                        nc.tensor.matmul(
                            scp[:, ts(h, QB)],
                            kT[(b, kc // 4)][
                                ds(h * DH, DH), ds((kc % 4) * P, P)
                            ],
                            qT[(b, qb)][ds(h * DH, DH), :],
                            start=True,
                            stop=True,
                            tile_position=(h * DH, 0),
                        )
                    pt = ptp.tile([P, NHL * QB], F16, tag="pT")
                    nc.scalar.activation(
                        pt, scp, mybir.ActivationFunctionType.Exp, scale=0.125
                    )
                    for h in range(NHL):
                        nc.tensor.matmul(
                            cps[h],
                            vaug[(b, h, kc)],
                            pt[:, ts(h, QB)],
                            start=(kc == 0),
                            stop=(kc == SKC - 1),
                        )
                # normalize + transpose out (xbar transpose, fp16 staging)
                out_tiles = [
                    outp.tile([P, FPC], F32, tag="out", name=f"out{i}")
                    for i in range(QB // P)
                ]
                for h in range(NHL):
                    csb = ctxsb.tile(
                        [CPAD, QB], F16, tag=f"csb{h}", name=f"csb{h}"
                    )
                    nc.vector.memset(csb, 1.0)
                    nc.vector.tensor_copy(csb[: DH + 1], cps[h])
                    for tb in range(QB // P):
                        pot = potp.tile([P, CPAD], F16, tag="pot", name="pot")
                        _xbar_or_fake(
                            nc, pot, csb[:, ts(tb, P)], qT[(0, 0)][:, :CPAD]
                        )
                        rec = smallp.tile([P, 1], F32, tag="rec", name="rec")
                        nc.vector.reciprocal(rec, pot[:, DH : DH + 1])
                        nc.vector.tensor_scalar_mul(
                            out_tiles[tb][:, ds(h * DH, DH)], pot[:, :DH], rec
                        )
                for tb in range(QB // P):
                    nc.sync.dma_start(
                        out_d[ds(b * S + qb * QB + tb * P, P), :],
                        out_tiles[tb],
                    )


_nc = None


def _get_program():
    global _nc
    if _nc is None:
        _nc = _build_program()
    return _nc


def kernel(
    hidden_states,
    attention_mask,
    Wq,
    bq,
    Wk,
    bk,
    Wv,
    bv,
):
    from concourse.bass_utils import run_bass_kernel_spmd

    hs = np.ascontiguousarray(np.asarray(hidden_states, np.float32)).reshape(
        T, HID
    )
    mask = np.asarray(attention_mask, np.float32)
    mask_t = np.ascontiguousarray(
        mask.reshape(B, SKC, P).transpose(2, 0, 1).reshape(P, B * SKC)
    )
    Wq, Wk, Wv = (np.asarray(w, np.float32) for w in (Wq, Wk, Wv))
    bq, bk, bv = (np.asarray(x, np.float32) for x in (bq, bk, bv))

    nc = _get_program()
    in_maps = []
    for c in range(NCORES):
        sl = slice(c * FPC, (c + 1) * FPC)
        in_maps.append(
            {
                "hs": hs,
                "wq": np.ascontiguousarray(Wq[sl]),
                "wk": np.ascontiguousarray(Wk[sl]),
                "wv": np.ascontiguousarray(Wv[sl]),
                "bq": np.ascontiguousarray(bq[sl]),
                "bk": np.ascontiguousarray(bk[sl]),
                "bv": np.ascontiguousarray(bv[sl]),
                "mk": mask_t,
            }
        )
    res = run_bass_kernel_spmd(nc, in_maps, list(range(NCORES))).results
    full = np.concatenate(
        [np.asarray(res[c]["out"]) for c in range(NCORES)], axis=1
    )
    return full.reshape(B, S, NH * DH).astype(np.float32)
